# revision 27
# baseline (speedup 1.0000x reference)
"""Self-contained distributed GAT kernel for 8 TRN2 NeuronCores (Bass/Tile).

Sharding: nodes (and incident edges, grouped by destination) across the
8 cores; weights replicated; per-layer feature tables exchanged via
AllGather; segment softmax/aggregation local per destination partition
in a rectangular [dst-row x edge-slot] layout filled by indexed DMA
gathers (4 parallel SWDGE queues, one per int16-addressable quarter).
Gathers use prepare_only + trigger_dma so the Pool engine does not hold
through the transfer: the four queues' transfers overlap. Padded slots
gather a reserved phantom "poison" row (s_src = -120) and vanish in the
softmax; aggregation is exp-weighted and normalized once per group.

kernel(**inputs) takes FULL inputs, returns (logits, probas) float32.
"""
import sys
import numpy as np

for _p in ('/opt/trn_rl_repo', '/root/.axon_site/_ro/trn_rl_repo'):
    if _p not in sys.path:
        sys.path.append(_p)

import concourse.bacc as bacc
from concourse import mybir, masks
from concourse.tile import TileContext
from concourse.bass_utils import run_bass_kernel_spmd
from contextlib import ExitStack

NCORES = 8
NQ = 4


def preprocess(edge_index, batch, N=100000, BLOCKS=98, NGRAPHS=256,
               CAPC=64, CAPQ=24, MAXSL=6):
    NPAD = NCORES * BLOCKS * 128
    SLAB = NPAD // NCORES
    QROWS = NPAD // 4
    GPC = NGRAPHS // NCORES
    loop = np.arange(N, dtype=np.int64)
    src = np.concatenate([loop, np.asarray(edge_index[0], dtype=np.int64)])
    dst = np.concatenate([loop, np.asarray(edge_index[1], dtype=np.int64)])
    batch = np.asarray(batch, dtype=np.int64)
    E = src.shape[0]

    deg = np.bincount(dst, minlength=N)   # includes self-loop

    # ---- pass 1: cores by total degree (snake), quarters FROZEN ----
    order = np.argsort(-deg, kind='stable')
    node_core = np.empty(N, np.int64)
    blk = np.arange(N) // 128
    s_, j_ = np.divmod(blk, NCORES)
    node_core[order] = np.where(s_ % 2 == 0, j_, NCORES - 1 - j_)
    node_quarter = node_core // 2
    degq = np.zeros((N, NQ), np.int64)
    np.add.at(degq, (dst, node_quarter[src]), 1)

    # ---- pass 2: per quarter-pair, rank-aligned by sorted per-quarter
    # degree shape (+argmax tiebreak) so same-slot nodes have similar
    # per-quarter maxima -> minimal rectangular padding ----
    M64 = 64
    srt = np.argsort(-degq, axis=1)
    dsort = -np.sort(-degq, axis=1)
    am, am2 = srt[:, 0], srt[:, 1]
    key = (((((dsort[:, 0] * 4 + am) * M64 + dsort[:, 1]) * 4 + am2) * M64
            + dsort[:, 2]) * M64 + dsort[:, 3])
    tpos = np.empty(N, np.int64)
    full = (BLOCKS - 1) * 256     # pair ranks in full slots 0..BLOCKS-2
    cap = full + 254
    plists = []
    for p in range(4):
        nodes = np.where(node_quarter == p)[0]
        plists.append(list(nodes[np.argsort(-key[nodes], kind='stable')]))
    for p in range(4):                      # spill overflow to emptiest pair
        while len(plists[p]) > cap:
            tgt = min(range(4), key=lambda i: len(plists[i]))
            assert len(plists[tgt]) < cap
            plists[tgt].append(plists[p].pop())
    for p in range(4):
        o2 = np.array(plists[p], dtype=np.int64)
        assert len(o2) <= cap, "pair overflow (poison row reservation)"
        r = np.arange(len(o2))
        slot = r // 256
        sub = r % 256
        core = 2 * p + (sub // 128)
        row = sub % 128
        # last slot: reserve row 127 on BOTH cores (phantom poison rows)
        tail = r >= full
        st = r[tail] - full
        slot[tail] = BLOCKS - 1
        core[tail] = 2 * p + (st >= 127)
        row[tail] = np.where(st >= 127, st - 127, st)
        tpos[o2] = core * SLAB + slot * 128 + row

    # ---- geometry ----
    d_t = tpos[dst]
    d_core = d_t // SLAB
    d_rem = d_t - d_core * SLAB
    d_slot = d_rem // 128
    d_row = d_rem - d_slot * 128
    s_t = tpos[src]
    s_q = s_t // QROWS
    s_local = (s_t - s_q * QROWS).astype(np.int16)

    cnt = np.zeros((NCORES, BLOCKS, 128, NQ), np.int64)
    np.add.at(cnt, (d_core, d_slot, d_row, s_q), 1)
    slot_q_max = np.maximum(cnt.max(axis=(0, 2)), 1)     # [BLOCKS, NQ]
    # greedy variable-size groups of consecutive slots, capped so that
    # nsl*sum(W) <= CAPC (combined-tile cols) and nsl*max(W) <= CAPQ
    # (per-quarter gather tile cols); heavy slots go alone, light tail
    # slots batch up (fewer gather calls)
    bounds = []
    s = 0
    while s < BLOCKS:
        n = 1
        while n < MAXSL and s + n < BLOCKS:
            W = slot_q_max[s:s + n + 1].max(axis=0)
            if (n + 1) * W.sum() > CAPC or (n + 1) * W.max() > CAPQ:
                break
            n += 1
        bounds.append((s, s + n))
        s += n
    NGRP = len(bounds)
    WG = np.zeros((NGRP, NQ), np.int64)
    GSL = np.zeros(NGRP, np.int64)                       # slots in group
    slot2grp = np.zeros(BLOCKS, np.int64)
    slot2lo = np.zeros(BLOCKS, np.int64)
    for g, (lo, hi) in enumerate(bounds):
        GSL[g] = hi - lo
        WG[g] = slot_q_max[lo:hi].max(axis=0)
        slot2grp[lo:hi] = g
        slot2lo[lo:hi] = lo
    d_grp = slot2grp[d_slot]
    d_sloc = d_slot - slot2lo[d_slot]
    # per-group quarter region starts; group total width
    qg0 = np.zeros((NGRP, NQ + 1), np.int64)
    for g in range(NGRP):
        qg0[g, 1:] = np.cumsum(WG[g] * GSL[g])
    GW = qg0[:, -1]                                      # group widths
    g0 = np.concatenate([[0], np.cumsum(GW)])            # group col starts
    WTOT = int(g0[-1])

    # rank within (core, slot, row, quarter)
    kk = ((d_core * BLOCKS + d_slot) * 128 + d_row) * NQ + s_q
    eorder = np.argsort(kk, kind='stable')
    ks = kk[eorder]
    first = np.concatenate([[True], ks[1:] != ks[:-1]])
    runstart = np.maximum.accumulate(np.where(first, np.arange(E), 0))
    rank = np.arange(E) - runstart

    lidx = np.full((NCORES, 128, WTOT), SLAB - 1, np.int16)  # poison row
    padmask = np.ones((NCORES, 128, WTOT), bool)
    eo = eorder
    col = (g0[d_grp[eo]] + qg0[d_grp[eo], s_q[eo]]
           + d_sloc[eo] * WG[d_grp[eo], s_q[eo]] + rank)
    lidx[d_core[eo], d_row[eo], col] = s_local[eo]
    padmask[d_core[eo], d_row[eo], col] = False
    assert (~padmask).sum() == E

    node_at = np.full((NCORES, SLAB), -1, np.int64)
    lp = tpos - (tpos // SLAB) * SLAB
    node_at[tpos // SLAB, lp] = np.arange(N)
    assert (node_at[:, SLAB - 1] == -1).all(), "last slab row must be phantom (poison row)"


    # ---- pooling: graph g -> (core, grow); its nodes on partitions
    # 4*grow + subrow (subrow = within-(graph,quarter) rank % 4) ----
    gsize = np.bincount(batch, minlength=NGRAPHS)
    gorder = np.argsort(-gsize, kind='stable')
    pool_core = np.empty(NGRAPHS, np.int64)
    pool_row = np.empty(NGRAPHS, np.int64)
    for i, g in enumerate(gorder):
        r, j = divmod(i, NCORES)
        pool_core[g] = j if r % 2 == 0 else NCORES - 1 - j
        pool_row[g] = r

    keyp = (pool_core[batch] * GPC + pool_row[batch]) * NQ + (tpos // QROWS)
    porder = np.argsort(keyp, kind='stable')
    kp = keyp[porder]
    firstp = np.concatenate([[True], kp[1:] != kp[:-1]])
    runstart = np.maximum.accumulate(np.where(firstp, np.arange(N), 0))
    rankp = np.arange(N) - runstart
    subrow = rankp % 4
    jcol = rankp // 4
    # per-quarter width = max over (core, graph) of ceil(count/4)
    pq = np.zeros((NCORES, GPC, NQ), np.int64)
    np.add.at(pq, (pool_core[batch], pool_row[batch], tpos // QROWS), 1)
    PWQS = np.maximum((pq + 3) // 4, 1).max(axis=(0, 1))   # [NQ]
    pq0 = np.concatenate([[0], np.cumsum(PWQS)])
    WPS = int(pq0[-1])

    pool_lidx = np.zeros((NCORES, 128, WPS), np.int16)
    pool_pad = np.ones((NCORES, 128, WPS), bool)
    pc = pool_core[batch][porder]
    pr = pool_row[batch][porder]
    ppart = pr * 4 + subrow
    pcol = pq0[(tpos // QROWS)[porder]] + jcol
    pool_lidx[pc, ppart, pcol] = (tpos - (tpos // QROWS) * QROWS)[porder].astype(np.int16)
    pool_pad[pc, ppart, pcol] = False
    pool_maskneg = np.where(pool_pad, np.float32(-1e30), np.float32(0.0))
    out_graph = np.empty((NCORES, GPC), np.int64)
    out_graph[pool_core, pool_row] = np.arange(NGRAPHS)

    return dict(
        tpos=tpos, node_at=node_at, BLOCKS=BLOCKS,
        lidx=lidx, WG=WG, GSL=GSL, qg0=qg0, g0=g0, WTOT=WTOT,
        pool_lidx=pool_lidx, pool_maskneg=pool_maskneg, PWQS=PWQS, pq0=pq0,
        WPS=WPS, out_graph=out_graph, deg=deg,
    )


def wrap_idx(vals):
    """[..., n] int16, n % 16 == 0: idx i -> [i%16, i//16], replicated x8 to
    128 partitions -> [..., 128, n/16]."""
    sh = vals.shape[:-1]
    n = vals.shape[-1]
    assert n % 16 == 0
    w = vals.reshape(*sh, n // 16, 16)
    w = np.swapaxes(w, -1, -2)
    w = np.broadcast_to(w[..., None, :, :], (*sh, 8, 16, n // 16))
    return w.reshape(*sh, 128, n // 16).copy()


def expand_a(a):
    heads, ch = a.shape
    A = np.zeros((heads * ch, heads), np.float32)
    for h in range(heads):
        A[h * ch:(h + 1) * ch, h] = a[h]
    return A




FP = mybir.dt.float32
BF = mybir.dt.bfloat16
I16 = mybir.dt.int16
ALU = mybir.AluOpType
ACTF = mybir.ActivationFunctionType
AX = mybir.AxisListType

NCORES = 8
NQ = 4
NL = 4
GPC = 32
NEG = 0.2
NEG_OUT = 0.01
HEADS = (4, 4, 4, 1)


def build(nc, geom):
    BLOCKS = int(geom["BLOCKS"])
    NP_ = NCORES * BLOCKS * 128
    SLAB = NP_ // NCORES
    QROWS = NP_ // NQ
    WG = np.asarray(geom["WG"])            # [NGRP, NQ]
    GSL = np.asarray(geom["GSL"])          # [NGRP]
    qg0 = np.asarray(geom["qg0"])          # [NGRP, NQ+1]
    g0 = np.asarray(geom["g0"])            # [NGRP+1]
    WTOT = int(geom["WTOT"])
    PWQS = np.asarray(geom["PWQS"])        # [NQ]
    pq0 = np.asarray(geom["pq0"])          # [NQ+1]
    WPS = int(geom["WPS"])
    NGRP = len(GSL)

    # ---------------- I/O ----------------
    xT = nc.declare_dram_parameter("xT", [128, SLAB], FP, isOutput=False)
    idx_in = nc.declare_dram_parameter("idx", [128, 8 * WTOT], I16, isOutput=False)
    pidx_in = nc.declare_dram_parameter("pool_idx", [128, 8 * WPS], I16, isOutput=False)
    pmask_in = nc.declare_dram_parameter("pool_maskneg", [128, WPS], FP, isOutput=False)
    wext_in = [
        nc.declare_dram_parameter(f"wext{l}", [128 if l == 0 else 64, 64 + 2 * HEADS[l]],
                                  FP, isOutput=False)
        for l in range(NL)
    ]
    bias_in = nc.declare_dram_parameter("bias", [128, NL, 64], FP, isOutput=False)
    fcW_in = nc.declare_dram_parameter("fcW", [64, 2], FP, isOutput=False)
    fcb_in = nc.declare_dram_parameter("fcb", [GPC, 2], FP, isOutput=False)
    logits_out = nc.declare_dram_parameter("logits", [GPC, 2], FP, isOutput=True)
    probas_out = nc.declare_dram_parameter("probas", [GPC, 2], FP, isOutput=True)

    with TileContext(nc) as tc, ExitStack() as ex:
        dram = ex.enter_context(tc.tile_pool(name="dram", bufs=1, space="DRAM"))
        tables = [dram.tile([NP_, 128], I16, addr_space="Shared", name=f"table{l}")
                  for l in range(NL + 1)]
        slabs = [dram.tile([SLAB, 128], I16, name=f"slab{l}") for l in range(NL + 1)]

        cpool = ex.enter_context(tc.tile_pool(name="const", bufs=1))
        gpools = [ex.enter_context(tc.tile_pool(name=f"gath{q}", bufs=6)) for q in range(NQ)]
        wpool = ex.enter_context(tc.tile_pool(name="wrk", bufs=3))
        xpool = ex.enter_context(tc.tile_pool(name="xin", bufs=3))
        rpool = ex.enter_context(tc.tile_pool(name="rows", bufs=3))
        ppool = ex.enter_context(tc.tile_pool(name="psum", bufs=4, space="PSUM"))
        ppool2 = ex.enter_context(tc.tile_pool(name="psum2", bufs=2, space="PSUM"))

        # hoisted constant registers: avoids a per-gather RegisterMove on the
        # Pool queue (those occupy exec-queue slots and halve DMA overlap)
        reg_cache = {}

        def creg(v):
            if v not in reg_cache:
                reg_cache[v] = nc.gpsimd.to_reg(v)
            return reg_cache[v]

        # ---- constants resident in SBUF ----
        ipool = ex.enter_context(tc.tile_pool(name="idxs", bufs=8))
        wext = []
        for l in range(NL):
            t = cpool.tile([128 if l == 0 else 64, 64 + 2 * HEADS[l]], FP,
                           name=f"wext_sb{l}")
            nc.sync.dma_start(t[:], wext_in[l][:])
            wext.append(t)
        bias_sb = cpool.tile([128, NL, 64], FP)
        nc.sync.dma_start(bias_sb[:], bias_in[:])
        ident = cpool.tile([128, 128], FP)
        masks.make_identity(nc, ident[:])
        poison_t = cpool.tile([1, 4], FP)
        nc.vector.memset(poison_t[:], -120.0)
        sdst_self = [cpool.tile([128, BLOCKS, 4], FP, name=f"sdst{i}") for i in range(2)]

        def matmul_to_row(l, s, lhsT_ap):
            """h_ext = lhsT.T @ wext[l] -> row [128,128] bf16 -> slab[l]; also
            stashes s_dst into sdst_self[l % 2]."""
            H = HEADS[l]
            pm = ppool.tile([128, 64 + 2 * H], FP, tag="mm", name=f"mm_{l}_{s}")
            nc.tensor.matmul(pm[:], lhsT_ap, wext[l][:], start=True, stop=True)
            row = rpool.tile([128, 128], I16, tag="row", name=f"row_{l}_{s}")
            nc.scalar.copy(row[:].bitcast(BF)[:, 0:64], pm[:, 0:64])
            rf = row[:].bitcast(FP)
            nc.vector.tensor_copy(rf[:, 32:32 + 2 * H], pm[:, 64:64 + 2 * H])
            # s_dst from the SBUF row (avoids a second PSUM read per block)
            nc.vector.tensor_copy(sdst_self[l % 2][:, s, 0:H], rf[:, 32 + H:32 + 2 * H])
            nc.vector.memset(rf[:, 32 + 2 * H:64], 0.0)
            nc.sync.dma_start(slabs[l][s * 128:(s + 1) * 128, :], row[:])
            if s == BLOCKS - 1:
                # poison row: phantom last slab row's s_src <- -120 so padded
                # gather slots vanish in the softmax (exp(leaky(-120+sdst))~0)
                nc.sync.dma_start(slabs[l][SLAB - 1:SLAB, 64:72],
                                  poison_t[:].bitcast(I16))

        # ---- layer-0 matmul phase ----
        for s in range(BLOCKS):
            xt = xpool.tile([128, 128], FP, tag="xt", name=f"xt0_{s}")
            nc.sync.dma_start(xt[:], xT[:, s * 128:(s + 1) * 128])
            matmul_to_row(0, s, xt[:])

        # ---- layers ----
        for l in range(NL):
            nc.gpsimd.collective_compute(
                "AllGather", ALU.bypass,
                ins=[slabs[l][:].opt()],
                outs=[tables[l][:].opt()],
                replica_groups=[list(range(NCORES))],
            )
            H = HEADS[l]
            ch = 64 // H
            # idx tiles prefetched PF groups ahead on the (idle) Scalar
            # engine's HWDGE queue so the lead gather never waits for them
            PF = 5
            itiles = {}

            def load_idx(gg):
                if gg >= NGRP:
                    return
                GWp = int(qg0[gg][NQ])
                t = ipool.tile([128, 8 * GWp], I16, tag="idx",
                               name=f"idx_{l}_{gg}")
                nc.scalar.dma_start(t[:], idx_in[:, 8 * int(g0[gg]):
                                            8 * int(g0[gg] + GWp)])
                itiles[gg] = t

            for gg in range(min(PF, NGRP)):
                load_idx(gg)
            for g in range(NGRP):
                lo = int(sum(GSL[:g]))
                nsl = int(GSL[g])
                GWg = int(qg0[g][NQ])
                itile = itiles.pop(g)
                load_idx(g + PF)
                Gq = []
                for q in range(NQ):
                    wq = int(WG[g][q])
                    ncols = nsl * wq
                    t = gpools[q].tile([128, ncols, 128], I16, tag=f"G{q}",
                                       name=f"G_{l}_{g}_{q}")
                    r0 = int(qg0[g][q])
                    nc.gpsimd.dma_gather(
                        t[:], tables[l][q * QROWS:(q + 1) * QROWS, :],
                        itile[:, 8 * r0: 8 * (r0 + ncols)],
                        128 * ncols, creg(128 * ncols), 128,
                        single_packet=False, queue_num=q)
                    Gq.append(t)
                # combined per-group tiles (ext/wt bf16, e fp32);
                # SW = per-slot total width across quarters
                SW = GWg // nsl
                e = wpool.tile([128, nsl, SW, H], FP, tag="e",
                               name=f"e_{l}_{g}")
                ext = wpool.tile([128, nsl, SW, H], BF, tag="ex",
                                 name=f"ex_{l}_{g}")
                wt = wpool.tile([128, nsl, SW, 64], BF, tag="wt",
                                name=f"wt_{l}_{g}")
                for q in range(NQ):
                    wq = int(WG[g][q])
                    off = int(qg0[g][q]) // nsl
                    Gf = Gq[q][:].bitcast(FP).rearrange("p (s j) e -> p s j e", s=nsl)
                    eq = e[:, :, off:off + wq, :]
                    nc.vector.tensor_tensor(
                        eq, Gf[:, :, :, 32:32 + H],
                        sdst_self[l % 2][:, lo:lo + nsl, 0:H]
                        .unsqueeze(2).broadcast_to([128, nsl, wq, H]),
                        ALU.add)
                    nc.vector.scalar_tensor_tensor(eq, eq, NEG, eq,
                                                   ALU.mult, ALU.max)
                    nc.scalar.activation(ext[:, :, off:off + wq, :], eq,
                                         ACTF.Exp)
                    # unnormalized weighted aggregation (releases Gq early)
                    nc.vector.tensor_tensor(
                        wt[:, :, off:off + wq, :]
                        .rearrange("p s j (h c) -> p s j h c", h=H),
                        Gq[q][:].bitcast(BF)[:, :, 0:64].rearrange(
                            "p (s j) (h c) -> p s j h c", s=nsl, h=H),
                        ext[:, :, off:off + wq, :].unsqueeze(4)
                        .broadcast_to([128, nsl, wq, H, ch]),
                        ALU.mult)
                den = wpool.tile([128, nsl, 4], FP, tag="den", name=f"den_{l}_{g}")
                nc.vector.tensor_reduce(
                    den[:, :, 0:H], ext[:].rearrange("p s j h -> p s h j"),
                    axis=AX.X, op=ALU.add)
                # in-place pairwise tree-fold over j (contiguous slices beat a
                # strided tensor_reduce); leftover odd column folds into col 0
                w = SW
                while w > 1:
                    h = w // 2
                    if w % 2:
                        nc.vector.tensor_tensor(wt[:, :, 0:1, :], wt[:, :, 0:1, :],
                                                wt[:, :, w - 1:w, :], ALU.add)
                    nc.vector.tensor_tensor(wt[:, :, 0:h, :], wt[:, :, 0:h, :],
                                            wt[:, :, h:2 * h, :], ALU.add)
                    w = h
                outg = wpool.tile([128, nsl, 64], FP, tag="outg", name=f"og_{l}_{g}")
                rden = wpool.tile([128, nsl, 4], FP, tag="rden", name=f"rd_{l}_{g}")
                nc.vector.reciprocal(rden[:, :, 0:H], den[:, :, 0:H])
                nc.vector.tensor_tensor(
                    outg[:].rearrange("p s (h c) -> p s h c", h=H),
                    wt[:, :, 0, :].rearrange("p s (h c) -> p s h c", h=H),
                    rden[:, :, 0:H].unsqueeze(3).broadcast_to([128, nsl, H, ch]),
                    ALU.mult)
                # bias + outer leaky for the whole group
                nc.vector.tensor_tensor(
                    outg[:], outg[:],
                    bias_sb[:, l, :].unsqueeze(1).broadcast_to([128, nsl, 64]),
                    ALU.add)
                nc.vector.scalar_tensor_tensor(outg[:], outg[:], NEG_OUT, outg[:],
                                               ALU.mult, ALU.max)
                for si in range(nsl):
                    s = lo + si
                    if l < NL - 1:
                        pt = ppool2.tile([64, 128], FP, tag="tp", name=f"tp_{l}_{s}")
                        nc.tensor.transpose(pt[:], outg[:, si, :], ident[:])
                        xtn = xpool.tile([64, 128], FP, tag="xtn", name=f"xtn_{l}_{s}")
                        nc.scalar.copy(xtn[:], pt[:])
                        matmul_to_row(l + 1, s, xtn[:])
                    else:
                        row = rpool.tile([128, 128], I16, tag="row", name=f"rowF_{s}")
                        rf = row[:].bitcast(FP)
                        nc.vector.tensor_copy(rf[:], outg[:, si, :])
                        nc.sync.dma_start(slabs[NL][s * 128:(s + 1) * 128, :], row[:])

        # ---- final AllGather (x_final fp32 rows) ----
        nc.gpsimd.collective_compute(
            "AllGather", ALU.bypass,
            ins=[slabs[NL][:].opt()],
            outs=[tables[NL][:].opt()],
            replica_groups=[list(range(NCORES))],
        )

        # ---- pooling ----
        pidx = cpool.tile([128, 8 * WPS], I16)
        nc.sync.dma_start(pidx[:], pidx_in[:])
        pmask = cpool.tile([128, WPS], FP)
        nc.sync.dma_start(pmask[:], pmask_in[:])
        pooled = cpool.tile([128, 64], FP)
        first = True
        PCH = 16
        for q in range(NQ):
            for k0 in range(0, int(PWQS[q]), PCH):
                wq = min(PCH, int(PWQS[q]) - k0)
                c0 = int(pq0[q]) + k0
                PG = gpools[q].tile([128, wq, 128], I16, tag=f"G{q}",
                                    name=f"PG_{q}_{k0}")
                nc.gpsimd.dma_gather(
                    PG[:], tables[NL][q * QROWS:(q + 1) * QROWS, :],
                    pidx[:, 8 * c0: 8 * (c0 + wq)],
                    128 * wq, creg(128 * wq), 128,
                    single_packet=False, queue_num=q)
                PGf = PG[:].bitcast(FP)                    # [128, wq, 64]
                pm = wpool.tile([128, wq, 64], BF, tag="pm", name=f"pm_{q}_{k0}")
                nc.vector.tensor_tensor(
                    pm[:], PGf,
                    pmask[:, c0:c0 + wq].unsqueeze(2).broadcast_to([128, wq, 64]),
                    ALU.add)
                red = wpool.tile([128, 64], FP, tag="red", name=f"red_{q}_{k0}")
                nc.vector.tensor_reduce(red[:], pm[:].rearrange("p w f -> p f w"),
                                        axis=AX.X, op=ALU.max)
                if first:
                    nc.vector.tensor_copy(pooled[:], red[:])
                    first = False
                else:
                    nc.vector.tensor_tensor(pooled[:], pooled[:], red[:], ALU.max)
        # transpose + fold the 4 subrows per graph via strided max-reduce
        ptp = ppool2.tile([64, 128], FP, tag="tp", name="pool_tp")
        nc.tensor.transpose(ptp[:], pooled[:], ident[:])
        ptps = cpool.tile([64, 128], FP)
        nc.scalar.copy(ptps[:], ptp[:])
        pooledT = cpool.tile([64, GPC], FP)
        nc.vector.tensor_reduce(
            pooledT[:], ptps[:].rearrange("p (g r) -> p g r", r=4),
            axis=AX.X, op=ALU.max)
        # FC + bias + softmax
        fcW = cpool.tile([64, 2], FP)
        nc.sync.dma_start(fcW[:], fcW_in[:])
        fcb = cpool.tile([GPC, 2], FP)
        nc.sync.dma_start(fcb[:], fcb_in[:])
        plog = ppool.tile([GPC, 2], FP, tag="mm", name="logits_mm")
        nc.tensor.matmul(plog[:], pooledT[:], fcW[:], start=True, stop=True)
        logits = cpool.tile([GPC, 2], FP)
        nc.vector.tensor_tensor(logits[:], plog[:], fcb[:], ALU.add)
        nc.sync.dma_start(logits_out[:], logits[:])
        m = cpool.tile([GPC, 1], FP)
        nc.vector.tensor_reduce(m[:], logits[:], axis=AX.X, op=ALU.max)
        z = cpool.tile([GPC, 2], FP)
        nc.vector.tensor_tensor(z[:], logits[:], m[:].broadcast_to([GPC, 2]),
                                ALU.subtract)
        ez = cpool.tile([GPC, 2], FP)
        nc.scalar.activation(ez[:], z[:], ACTF.Exp)
        den2 = cpool.tile([GPC, 1], FP)
        nc.vector.tensor_reduce(den2[:], ez[:], axis=AX.X, op=ALU.add)
        rden2 = cpool.tile([GPC, 1], FP)
        nc.vector.reciprocal(rden2[:], den2[:])
        probas = cpool.tile([GPC, 2], FP)
        nc.vector.tensor_tensor(probas[:], ez[:], rden2[:].broadcast_to([GPC, 2]),
                                ALU.mult)
        nc.sync.dma_start(probas_out[:], probas[:])
    return nc


def make_inputs(P, inp):
    """Per-core in_maps from preprocess() result P and problem inputs."""
    BLOCKS = int(P["BLOCKS"])
    SLAB = BLOCKS * 128
    x = np.asarray(inp["x"], np.float32)
    F = x.shape[1]
    wext_np = []
    for l in range(NL):
        Wl = np.asarray(inp[f"W{l+1}"], np.float32)
        As = expand_a(np.asarray(inp[f"a{l+1}s"], np.float32))
        Ad = expand_a(np.asarray(inp[f"a{l+1}d"], np.float32))
        wext_np.append(np.concatenate([Wl, Wl @ As, Wl @ Ad], axis=1))
    bias_np = np.stack([np.asarray(inp[f"b{l+1}"], np.float32) for l in range(NL)])
    bias_rep = np.tile(bias_np[None], (128, 1, 1))
    fcW = np.asarray(inp["fcW"], np.float32)
    fcb = np.tile(np.asarray(inp["fcb"], np.float32)[None, :], (GPC, 1))

    # wrapped idx: per gather region (column range), stream = col-major
    WG, GSL, qg0, g0 = P["WG"], P["GSL"], P["qg0"], P["g0"]
    NGRP = len(GSL)
    regions = []
    for g in range(NGRP):
        for q in range(NQ):
            c0 = int(g0[g] + qg0[g][q])
            regions.append((c0, int(GSL[g]) * int(WG[g][q])))
    pregions = [(int(P["pq0"][q]), int(P["PWQS"][q])) for q in range(NQ)]

    def build_idx(lidx_c, regs):
        parts = []
        for c0, ncols in regs:
            stream = lidx_c[:, c0:c0 + ncols].T.reshape(1, -1)   # col-major
            parts.append(wrap_idx(stream)[0])
        return np.concatenate(parts, axis=1).astype(np.int16)

    in_maps = []
    for c in range(NCORES):
        nodes = P["node_at"][c]
        xs = np.zeros((SLAB, F), np.float32)
        valid = nodes >= 0
        xs[valid] = x[nodes[valid]]
        m = {
            "xT": np.ascontiguousarray(xs.T),
            "idx": build_idx(P["lidx"][c], regions),
            "pool_idx": build_idx(P["pool_lidx"][c], pregions),
            "pool_maskneg": P["pool_maskneg"][c].astype(np.float32),
            "bias": bias_rep, "fcW": fcW, "fcb": fcb,
        }
        for l in range(NL):
            m[f"wext{l}"] = wext_np[l]
        in_maps.append(m)
    return in_maps


def _run(inputs, trace=False):
    inp = {k: np.asarray(v) for k, v in inputs.items()}
    P = preprocess(inp['edge_index'], inp['batch'], N=100000, BLOCKS=98,
                   NGRAPHS=256)
    in_maps = make_inputs(P, inp)
    nc = bacc.Bacc("TRN2", num_swdge_queues=4)
    build(nc, P)
    nc.compile()
    res = run_bass_kernel_spmd(nc, in_maps, list(range(NCORES)), trace=trace)
    logits = np.zeros((256, 2), np.float32)
    probas = np.zeros((256, 2), np.float32)
    for c in range(NCORES):
        lg = res.results[c]["logits"]
        pb = res.results[c]["probas"]
        for r in range(GPC):
            g = P["out_graph"][c, r]
            logits[g] = lg[r]
            probas[g] = pb[r]
    return logits, probas, res.exec_time_ns


def kernel(**inputs):
    logits, probas, _ = _run(inputs, trace=False)
    return logits, probas
